# revision 1
# baseline (speedup 1.0000x reference)
"""HGConv fused kernel for one TRN2 chip (8 NeuronCores), SPMD via Bass/Tile.

Hardcoded for M=16384 nodes, E=4096 hyperedges, D=300, N_CAT=3, 8 cores.

  - Shard the node axis m: core c gets node_feats rows [2048c, 2048(c+1))
    and the matching inc_mat rows.  Phase 1 computes the partial
    IX_c = inc_c.T @ X_c (4096, 300) with inc tiles stationary on the PE.
  - ReduceScatter(add) turns the partials into the true IX = inc.T @ X,
    e-sharded: core c owns edges [512c, 512(c+1)).
  - Local tail per core: edge_att = IX @ W_att (reassociated from
    inc.T @ (X @ W_att)), softmax over d, ef = (IX * attn) @ W_proj,
    residual mix with edge_feats, scores = ef2 @ ec_W_att, locally
    stabilized exp, G = ef2 @ ec_W_proj, partial pooled vector
    p2 = sum_e exp_e * G[e, :].
  - AllGather of the per-core (p2, z, m) partials (304 floats); every core
    redundantly combines them (global softmax over edges) and applies the
    two tiny projections to produce the (3,) logits.
"""

import sys

for _p in ("/opt/trn_rl_repo", "/opt/pypackages"):
    if _p not in sys.path:
        sys.path.append(_p)

import numpy as np

import concourse.bacc as bacc
import concourse.tile as tile
from concourse import masks, mybir
from concourse.bass_utils import run_bass_kernel_spmd

F32 = mybir.dt.float32
F32R = mybir.dt.float32r
BF16 = mybir.dt.bfloat16
AX = mybir.AxisListType
OP = mybir.AluOpType
AF = mybir.ActivationFunctionType

NCORES = 8
M, E, D, NCAT = 16384, 4096, 300, 3
M_SH = M // NCORES          # 2048 nodes per core
E_SH = E // NCORES          # 512 edges per core (tail shard)
MT = M_SH // 128            # 16 m-tiles per core
ET_SH = E_SH // 128         # 4 e-tiles per core
DCH = (128, 128, 44)        # d split into partition chunks
DOF = (0, 128, 256)
E_BLK = 1024                # phase-1 e block (8 psum banks)
N_EBLK = E // E_BLK
E_SUB = E_BLK // 128


def _build(alpha: float, mode: str):
    nc = bacc.Bacc("TRN2", target_bir_lowering=False, debug=False,
                   num_devices=NCORES)
    in_dt = BF16 if mode == "bf16" else F32
    x_d = nc.dram_tensor("x", [M_SH, D], in_dt, kind="ExternalInput")
    inc_d = nc.dram_tensor("inc", [M_SH, E], in_dt, kind="ExternalInput")
    ef_d = nc.dram_tensor("efeat", [E_SH, D], F32, kind="ExternalInput")
    watt_d = nc.dram_tensor("watt", [D, D], F32, kind="ExternalInput")
    wproj_d = nc.dram_tensor("wproj", [D, D], F32, kind="ExternalInput")
    ecwatt_d = nc.dram_tensor("ecwatt", [D, 1], F32, kind="ExternalInput")
    ecwproj_d = nc.dram_tensor("ecwproj", [D, D], F32, kind="ExternalInput")
    ecb_d = nc.dram_tensor("ecb", [D], F32, kind="ExternalInput")
    fcw_d = nc.dram_tensor("fcw", [D, NCAT], F32, kind="ExternalInput")
    fcb_d = nc.dram_tensor("fcb", [NCAT], F32, kind="ExternalInput")
    out_d = nc.dram_tensor("out", [1, NCAT], F32, kind="ExternalOutput")

    groups = [list(range(NCORES))]

    rdt = {"f32": F32, "f32r": F32R, "bf16": BF16}[mode]
    e_blk = 2048 if mode == "bf16" else 1024
    n_eblk = E // e_blk
    e_sub = e_blk // 128

    def mm(out, lhsT, rhs, start, stop):
        nc.tensor.matmul(out, lhsT, rhs, start=start, stop=stop)

    def rsrc(ap):
        return ap.bitcast(F32R) if mode == "f32r" else ap

    with tile.TileContext(nc) as tc, \
         tc.tile_pool(name="sb", bufs=1) as sb, \
         tc.tile_pool(name="dram", bufs=1, space="DRAM") as dram:

        p_chunks = [dram.tile([1024, D], F32, name=f"p_chunk{k}")
                    for k in range(4)]          # RS inputs (partial IX)
        r_ks = [dram.tile([128, D], F32, name=f"r_k{k}")
                for k in range(4)]              # RS outputs (my 128 edges)
        pk_dram = dram.tile([304], F32)         # AG input
        gath = dram.tile([NCORES, 304], F32)    # AG output

        # ---------- phase 1: IX partial = inc_c.T @ X_c ----------
        x_sb = sb.tile([128, MT, D], rdt)
        nc.sync.dma_start(x_sb[:], rsrc(x_d.ap().rearrange("(t p) d -> p t d",
                                                           p=128)))
        with tc.tile_pool(name="incp", bufs=MT + 8) as incp, \
             tc.tile_pool(name="stg", bufs=8) as stg, \
             tc.tile_pool(name="pp1", bufs=8, space="PSUM") as pp1:
            for blk in range(n_eblk):
                inc_sb = [incp.tile([128, e_blk], rdt, tag="inc",
                                    name=f"inc_b{blk}_m{m}")
                          for m in range(MT)]
                for m in range(MT):
                    eng = nc.sync if m % 2 == 0 else nc.scalar
                    eng.dma_start(
                        inc_sb[m][:],
                        rsrc(inc_d[m * 128:(m + 1) * 128,
                                   blk * e_blk:(blk + 1) * e_blk]))
                for es in range(e_sub):
                    acc = pp1.tile([128, D], F32, tag="p1")
                    for m in range(MT):
                        mm(acc[:], inc_sb[m][:, es * 128:(es + 1) * 128],
                           x_sb[:, m, :], start=(m == 0), stop=(m == MT - 1))
                    stage = stg.tile([128, D], F32, tag="stage",
                                     name=f"stage_{blk}_{es}")
                    nc.vector.tensor_copy(stage[:], acc[:])
                    eg = blk * e_blk + es * 128        # global edge offset
                    k, row = eg // 1024, eg % 1024
                    nc.gpsimd.dma_start(p_chunks[k][row:row + 128, :],
                                        stage[:])
                    # phase 2 (chunked, overlapped): as soon as chunk k is
                    # fully written, ReduceScatter it while the next block
                    # computes.
                    if row == 1024 - 128:
                        nc.gpsimd.collective_compute(
                            "ReduceScatter", OP.add, replica_groups=groups,
                            ins=[p_chunks[k].opt()], outs=[r_ks[k].opt()])

        # ---------- small weights / constants ----------
        watt_sb = sb.tile([128, 3, D], F32)
        wproj_sb = sb.tile([128, 3, D], F32)
        ecwproj_sb = sb.tile([128, 3, D], F32)
        fcw_sb = sb.tile([128, 3, NCAT], F32)
        ecwatt_sb = sb.tile([128, 3, 1], F32)
        for i, (c, o) in enumerate(zip(DCH, DOF)):
            nc.sync.dma_start(watt_sb[:c, i, :], watt_d[o:o + c, :])
            nc.sync.dma_start(wproj_sb[:c, i, :], wproj_d[o:o + c, :])
            nc.sync.dma_start(ecwproj_sb[:c, i, :], ecwproj_d[o:o + c, :])
            nc.sync.dma_start(fcw_sb[:c, i, :], fcw_d[o:o + c, :])
            nc.sync.dma_start(ecwatt_sb[:c, i, :], ecwatt_d[o:o + c, :])
        ecb_sb = sb.tile([1, D], F32)
        nc.sync.dma_start(ecb_sb[:], ecb_d.ap().rearrange("(o d) -> o d", o=1))
        fcb_sb = sb.tile([1, NCAT], F32)
        nc.sync.dma_start(fcb_sb[:], fcb_d.ap().rearrange("(o d) -> o d", o=1))
        ident = sb.tile([128, 128], F32)
        masks.make_identity(nc, ident[:])
        efeat_sb = sb.tile([128, ET_SH, D], F32)
        nc.sync.dma_start(efeat_sb[:],
                          ef_d.ap().rearrange("(t p) d -> p t d", p=128))

        # ---------- phase 3: local tail on this core's 512 edges ----------
        ix_sb = sb.tile([128, ET_SH, D], F32)
        for k in range(4):
            nc.sync.dma_start(ix_sb[:, k, :], r_ks[k][:])

        with tc.tile_pool(name="pp2", bufs=4, space="PSUM") as pp:

            def transpose_512xD(src_sb, dstT_sb):
                # src (128, 4, 300) [e-part] -> dstT (128, 3, 512) [d-part]
                for et in range(ET_SH):
                    for i, (c, o) in enumerate(zip(DCH, DOF)):
                        tp = pp.tile([128, 128], F32, tag="ps")
                        nc.tensor.transpose(tp[:c, :128],
                                            src_sb[:, et, o:o + c], ident[:])
                        nc.scalar.copy(
                            dstT_sb[:c, i, et * 128:(et + 1) * 128],
                            tp[:c, :128])

            ixT_sb = sb.tile([128, 3, E_SH], F32)
            transpose_512xD(ix_sb, ixT_sb)

            # edge_att = IX @ W_att; softmax over d; ef = IX * attn
            ef2_sb = sb.tile([128, ET_SH, D], F32)
            stat_sb = sb.tile([128, ET_SH, 4], F32)
            for et in range(ET_SH):
                att = pp.tile([128, D], F32, tag="ps")
                for i, c in enumerate(DCH):
                    mm(att[:], ixT_sb[:c, i, et * 128:(et + 1) * 128],
                       watt_sb[:c, i, :], start=(i == 0), stop=(i == 2))
                nmax = stat_sb[:, et, 0:1]
                nc.vector.tensor_reduce(nmax, att[:], axis=AX.X, op=OP.max,
                                        negate=True)
                ex = pp.tile([128, D], F32, tag="ps")
                rsum = stat_sb[:, et, 1:2]
                nc.scalar.activation(ex[:], att[:], AF.Exp, bias=nmax,
                                     scale=1.0, accum_out=rsum)
                rcp = stat_sb[:, et, 2:3]
                nc.vector.reciprocal(rcp, rsum)
                nc.vector.scalar_tensor_tensor(
                    ef2_sb[:, et, :], ex[:], rcp, ix_sb[:, et, :],
                    op0=OP.mult, op1=OP.mult)

            efT_sb = sb.tile([128, 3, E_SH], F32)
            transpose_512xD(ef2_sb, efT_sb)

            # ef2 = alpha * edge_feats + (1 - alpha) * (ef @ W_proj)
            efs_sb = sb.tile([128, ET_SH, D], F32)
            for et in range(ET_SH):
                prj = pp.tile([128, D], F32, tag="ps")
                for i, c in enumerate(DCH):
                    mm(prj[:], efT_sb[:c, i, et * 128:(et + 1) * 128],
                       wproj_sb[:c, i, :], start=(i == 0), stop=(i == 2))
                nc.scalar.mul(efs_sb[:, et, :], efeat_sb[:, et, :],
                              float(alpha))
                nc.vector.scalar_tensor_tensor(
                    ef2_sb[:, et, :], prj[:], float(1.0 - alpha),
                    efs_sb[:, et, :], op0=OP.mult, op1=OP.add)

            ef2T_sb = sb.tile([128, 3, E_SH], F32)
            transpose_512xD(ef2_sb, ef2T_sb)

            # scores (1, 512); locally stabilized exp weights
            sc = pp.tile([1, E_SH], F32, tag="ps")
            for i, c in enumerate(DCH):
                mm(sc[:], ecwatt_sb[:c, i, :], ef2T_sb[:c, i, :],
                   start=(i == 0), stop=(i == 2))
            one_sb = sb.tile([1, 520], F32)
            nloc = one_sb[:, 512:513]
            nc.vector.tensor_reduce(nloc, sc[:], axis=AX.X, op=OP.max,
                                    negate=True)
            expw = one_sb[:, 0:512]
            zloc = one_sb[:, 513:514]
            nc.scalar.activation(expw, sc[:], AF.Exp, bias=nloc, scale=1.0,
                                 accum_out=zloc)
            mloc = one_sb[:, 514:515]
            nc.scalar.mul(mloc, nloc, -1.0)

            expcol_sb = sb.tile([128, ET_SH], F32)
            for et in range(ET_SH):
                tc1 = pp.tile([128, 1], F32, tag="ps")
                nc.tensor.transpose(tc1[:],
                                    expw[0:1, et * 128:(et + 1) * 128],
                                    ident[0:1, 0:1])
                nc.scalar.copy(expcol_sb[:, et:et + 1], tc1[:])

            # G = ef2 @ ec_W_proj ; p2 = expw^T @ G (pooling + proj folded)
            g_sb = sb.tile([128, ET_SH, D], F32)
            for et in range(ET_SH):
                g = pp.tile([128, D], F32, tag="ps")
                for i, c in enumerate(DCH):
                    mm(g[:], ef2T_sb[:c, i, et * 128:(et + 1) * 128],
                       ecwproj_sb[:c, i, :], start=(i == 0), stop=(i == 2))
                nc.scalar.copy(g_sb[:, et, :], g[:])
            p2 = pp.tile([1, D], F32, tag="acc")
            for et in range(ET_SH):
                mm(p2[:], expcol_sb[:, et:et + 1], g_sb[:, et, :],
                   start=(et == 0), stop=(et == ET_SH - 1))

            pk_sb = sb.tile([1, 304], F32)
            nc.scalar.copy(pk_sb[:, 0:D], p2[:])
            nc.scalar.copy(pk_sb[:, 300:301], zloc)
            nc.scalar.copy(pk_sb[:, 301:302], mloc)
            nc.vector.memset(pk_sb[:, 302:304], 0.0)
            nc.sync.dma_start(pk_dram[:], pk_sb[0:1, :])

            # ---------- phase 4: AllGather + redundant epilogue ----------
            nc.gpsimd.collective_compute(
                "AllGather", OP.bypass, replica_groups=groups,
                ins=[pk_dram.opt()], outs=[gath.opt()])

            grow = sb.tile([1, NCORES, 304], F32)
            nc.sync.dma_start(
                grow[:], gath[:].rearrange("c k -> (c k)").rearrange(
                    "(o c k) -> o c k", o=1, c=NCORES))
            g8 = sb.tile([NCORES, 304], F32)
            nc.sync.dma_start(g8[:], gath[:])

            eps_sb = sb.tile([1, 16], F32)
            ngmax = eps_sb[:, 0:1]
            nc.vector.tensor_reduce(ngmax, grow[:, :, 301], axis=AX.X,
                                    op=OP.max, negate=True)
            scal_row = eps_sb[:, 1:9]
            nc.scalar.activation(scal_row, grow[:, :, 301], AF.Exp,
                                 bias=ngmax, scale=1.0)
            sccol = pp.tile([NCORES, 1], F32, tag="ps")
            nc.tensor.transpose(sccol[:], scal_row, ident[0:1, 0:1])
            sccol_sb = sb.tile([NCORES, 1], F32)
            nc.scalar.copy(sccol_sb[:], sccol[:])
            comb = pp.tile([1, 304], F32, tag="ps")
            nc.tensor.matmul(comb[:], sccol_sb[:], g8[:], start=True,
                             stop=True)
            rz = eps_sb[:, 9:10]
            nc.vector.reciprocal(rz, comb[:, 300:301])
            pooled_sb = sb.tile([1, D], F32)
            nc.vector.tensor_scalar_mul(pooled_sb[:], comb[:, 0:D], rz)
            nc.vector.tensor_add(pooled_sb[:], pooled_sb[:], ecb_sb[:])

            ocol_sb = sb.tile([128, 3], F32)
            for i, (c, o) in enumerate(zip(DCH, DOF)):
                tpc = pp.tile([128, 1], F32, tag="ps")
                nc.tensor.transpose(tpc[:c, :], pooled_sb[0:1, o:o + c],
                                    ident[0:1, 0:1])
                nc.scalar.copy(ocol_sb[:c, i:i + 1], tpc[:c, :])
            lg = pp.tile([1, NCAT], F32, tag="acc")
            for i, c in enumerate(DCH):
                nc.tensor.matmul(lg[:], ocol_sb[:c, i:i + 1],
                                 fcw_sb[:c, i, :], start=(i == 0),
                                 stop=(i == 2))
            logit_sb = sb.tile([1, NCAT], F32)
            nc.vector.tensor_add(logit_sb[:], lg[:], fcb_sb[:])
            nc.sync.dma_start(out_d[:], logit_sb[:])

    nc.compile()
    return nc


_CACHE = {}


def get_nc(alpha: float, mode: str = "f32r"):
    key = (alpha, mode)
    if key not in _CACHE:
        _CACHE[key] = _build(alpha, mode)
    return _CACHE[key]


def make_in_maps(node_feats, edge_feats, inc_mat, W_att, W_proj,
                 ec_W_att, ec_W_proj, ec_b_proj, fc_W, fc_b, mode="f32r"):
    cc = lambda a: np.ascontiguousarray(np.asarray(a, np.float32))
    node_feats, inc_mat, edge_feats = cc(node_feats), cc(inc_mat), cc(edge_feats)
    if mode == "bf16":
        import ml_dtypes
        node_feats = node_feats.astype(ml_dtypes.bfloat16)
        inc_mat = inc_mat.astype(ml_dtypes.bfloat16)
    common = dict(watt=cc(W_att), wproj=cc(W_proj),
                  ecwatt=cc(ec_W_att).reshape(D, 1), ecwproj=cc(ec_W_proj),
                  ecb=cc(ec_b_proj), fcw=cc(fc_W), fcb=cc(fc_b))
    in_maps = []
    for c in range(NCORES):
        # under chunked RS, core c owns edges {1024k + 128c .. +128} k=0..3
        eidx = np.concatenate([np.arange(1024 * k + 128 * c,
                                         1024 * k + 128 * (c + 1))
                               for k in range(4)])
        in_maps.append(dict(
            x=node_feats[c * M_SH:(c + 1) * M_SH],
            inc=np.ascontiguousarray(inc_mat[c * M_SH:(c + 1) * M_SH]),
            efeat=np.ascontiguousarray(edge_feats[eidx]),
            **common))
    return in_maps


def kernel(node_feats, edge_feats, inc_mat, W_att, W_proj, alpha,
           ec_W_att, ec_W_proj, ec_b_proj, fc_W, fc_b,
           mode="f32r", trace=False):
    nc = get_nc(float(np.asarray(alpha)), mode)
    in_maps = make_in_maps(node_feats, edge_feats, inc_mat, W_att, W_proj,
                           ec_W_att, ec_W_proj, ec_b_proj, fc_W, fc_b,
                           mode=mode)
    res = run_bass_kernel_spmd(nc, in_maps, list(range(NCORES)), trace=trace)
    kernel.last_results = res
    return res.results[0]["out"].reshape(NCAT).astype(np.float32)



# revision 10
# speedup vs baseline: 1.0451x; 1.0451x over previous
"""HGConv fused kernel for one TRN2 chip (8 NeuronCores), SPMD via Bass/Tile.

Hardcoded for M=16384 nodes, E=4096 hyperedges, D=300, N_CAT=3, 8 cores.

Edge-sharded design (v2) — no mid-kernel collectives:
  - Core c owns edges Ec = [512c, 512(c+1)).  Inputs per core: full X
    (bf16), inc[:, Ec] (bf16, host-sliced contiguous), edge_feats[Ec].T
    (f32, host-transposed), and the small weights.
  - Phase 1 computes IX_c = inc_c.T @ X over ALL 16384 nodes with bf16
    matmuls into 4 psum banks (one per 128-edge sub-block), es-major so
    sub-blocks complete progressively and the tail pipelines behind.
  - Tail per 128-edge block: transpose IX -> d-partitioned, edge_att =
    IX @ W_att (f32r), rowwise softmax over d, ef = attn * IX,
    transpose ef, ef2T = alpha*edge_featsT + (1-alpha)*(W_proj.T@efT),
    scores = ec_W_att.T @ ef2T (no stabilization needed: |scores| < 5),
    G2 = ef2 @ (ec_W_proj @ fc_W)  [W2 precomputed on device],
    p2 = expw.T @ G2, z = sum(expw).
  - One 4-float AllReduce(add) of [p2, z]; every core then emits
    logits = p2/z + (ec_b_proj @ fc_W + fc_b).
"""

import sys

for _p in ("/opt/trn_rl_repo", "/opt/pypackages"):
    if _p not in sys.path:
        sys.path.append(_p)

import numpy as np

import concourse.bacc as bacc
import concourse.tile as tile
from concourse import masks, mybir
from concourse.bass_utils import run_bass_kernel_spmd

F32 = mybir.dt.float32
F32R = mybir.dt.float32r
BF16 = mybir.dt.bfloat16
AX = mybir.AxisListType
OP = mybir.AluOpType
AF = mybir.ActivationFunctionType

NCORES = 8
M, E, D, NCAT = 16384, 4096, 300, 3
E_SH = E // NCORES          # 512 edges per core
ET = E_SH // 128            # 4 e-sub-blocks per core
MT = M // 128               # 128 m-tiles
MH = MT // 2                # 64 m-tiles per half
DCH = (128, 128, 44)        # d split into partition chunks
DOF = (0, 128, 256)


def _build(alpha: float):
    nc = bacc.Bacc("TRN2", target_bir_lowering=False, debug=False,
                   num_devices=NCORES)
    x_d = nc.dram_tensor("x", [M, D], BF16, kind="ExternalInput")
    inc_d = nc.dram_tensor("inc", [M, E_SH], BF16, kind="ExternalInput")
    eft_d = nc.dram_tensor("eft", [D, E_SH], F32, kind="ExternalInput")
    watt_d = nc.dram_tensor("watt", [D, D], F32, kind="ExternalInput")
    wproj_d = nc.dram_tensor("wproj", [D, D], F32, kind="ExternalInput")
    ecwatt_d = nc.dram_tensor("ecwatt", [D, 1], F32, kind="ExternalInput")
    # ec_W_proj passed TRANSPOSED from host (only used via W2 = ecp @ fcw)
    ecpT_d = nc.dram_tensor("ecpt", [D, D], F32, kind="ExternalInput")
    ecb_d = nc.dram_tensor("ecb", [D], F32, kind="ExternalInput")
    fcw_d = nc.dram_tensor("fcw", [D, NCAT], F32, kind="ExternalInput")
    fcb_d = nc.dram_tensor("fcb", [NCAT], F32, kind="ExternalInput")
    out_d = nc.dram_tensor("out", [1, NCAT], F32, kind="ExternalOutput")

    groups = [list(range(NCORES))]

    def r(ap):
        return ap.bitcast(F32R)

    with tile.TileContext(nc) as tc, \
         tc.tile_pool(name="sb", bufs=1) as sb, \
         tc.tile_pool(name="dram", bufs=1, space="DRAM") as dram:

        prt_d = dram.tile([4], F32)       # AllReduce input  [p2, z]
        cmb_d = dram.tile([4], F32)       # AllReduce output

        # ---------- small weight loads (DVE + Pool rings) ----------
        watt_sb = sb.tile([128, 3, D], F32)
        wproj_sb = sb.tile([128, 3, D], F32)
        ecpT_sb = sb.tile([128, 3, D], F32)
        fcw_sb = sb.tile([128, 3, NCAT], F32)
        ecwatt_sb = sb.tile([128, 3, 1], F32)
        ecbc_sb = sb.tile([128, 3, 1], F32)
        eft_sb = sb.tile([128, 3, E_SH], F32)
        efs_sb = sb.tile([128, 3, E_SH], F32)
        for i, (c, o) in enumerate(zip(DCH, DOF)):
            nc.scalar.dma_start(watt_sb[:c, i, :].bitcast(F32R),
                                watt_d[o:o + c, :].bitcast(F32R))
            nc.scalar.dma_start(wproj_sb[:c, i, :].bitcast(F32R),
                                wproj_d[o:o + c, :].bitcast(F32R))
            nc.scalar.dma_start(eft_sb[:c, i, :], eft_d[o:o + c, :])
            nc.gpsimd.dma_start(ecpT_sb[:c, i, :], ecpT_d[o:o + c, :])
            nc.gpsimd.dma_start(fcw_sb[:c, i, :], fcw_d[o:o + c, :])
            nc.gpsimd.dma_start(ecwatt_sb[:c, i, :].bitcast(F32R),
                                ecwatt_d[o:o + c, :].bitcast(F32R))
            nc.gpsimd.dma_start(
                ecbc_sb[:c, i, 0:1],
                ecb_d[o:o + c].rearrange("(p o) -> p o", o=1))
        fcb_sb = sb.tile([1, NCAT], F32)
        nc.gpsimd.dma_start(fcb_sb[:], fcb_d.ap().rearrange("(o d) -> o d",
                                                            o=1))
        ident = sb.tile([128, 128], F32)
        masks.make_identity(nc, ident[:])

        # efs = alpha * edge_feats.T  (d-partitioned), overlapped w/ phase 1
        for i, (c, o) in enumerate(zip(DCH, DOF)):
            nc.scalar.mul(efs_sb[:c, i, :], eft_sb[:c, i, :], float(alpha))

        # ---------- phase 1 inputs on the sync ring (ordered) ----------
        x_sb = sb.tile([128, MT, D], BF16)
        x_re = x_d.ap().rearrange("(t p) d -> p t d", p=128)
        inc_re = inc_d.ap().rearrange("(t p) e -> p t e", p=128)

        # persistent tail state
        ixT_sb = sb.tile([128, 3, E_SH], F32)
        efT_sb = sb.tile([128, 3, E_SH], F32)
        ef2T_sb = sb.tile([128, 3, E_SH], F32)
        w2_sb = sb.tile([128, 3, NCAT], F32)
        stat_sb = sb.tile([128, ET, 4], F32)
        expw_sb = sb.tile([1, E_SH + 4], F32)
        expcol_sb = sb.tile([128, ET], F32)
        g2_sb = sb.tile([128, ET, NCAT], F32)
        b2_sb = sb.tile([1, 4], F32)
        prt_sb = sb.tile([1, 4], F32)
        cmb_sb = sb.tile([1, 4], F32)
        logit_sb = sb.tile([1, NCAT], F32)

        with tc.tile_pool(name="incp", bufs=3) as incp, \
             tc.tile_pool(name="ixp", bufs=2) as ixp, \
             tc.tile_pool(name="exp", bufs=2) as exp_p, \
             tc.tile_pool(name="efp", bufs=2) as efp, \
             tc.tile_pool(name="pp1", bufs=2, space="PSUM") as pp1, \
             tc.tile_pool(name="ppt", bufs=2, space="PSUM") as ppt, \
             tc.tile_pool(name="ppa", bufs=1, space="PSUM") as ppa, \
             tc.tile_pool(name="ppj", bufs=2, space="PSUM") as ppj, \
             tc.tile_pool(name="pps", bufs=1, space="PSUM") as pps:

            # DMA issue order on the sync ring defines the byte schedule:
            # x interleaved with inc columns, es-major, so e-sub-blocks
            # complete progressively.
            inc_t = [[None, None] for _ in range(ET)]
            for es in range(ET):
                inc_t[es][0] = incp.tile([128, MH, 128], BF16, tag="inc",
                                         name=f"i{es}0")
                inc_t[es][1] = incp.tile([128, MH, 128], BF16, tag="inc",
                                         name=f"i{es}1")
            order = [("x", 0), ("i", 0, 0), ("x", 1), ("i", 0, 1),
                     ("x", 2), ("i", 1, 0), ("x", 3), ("i", 1, 1),
                     ("i", 2, 0), ("i", 2, 1), ("i", 3, 0), ("i", 3, 1)]
            for item in order:
                if item[0] == "x":
                    k = item[1]
                    nc.sync.dma_start(x_sb[:, 32 * k:32 * (k + 1), :],
                                      x_re[:, 32 * k:32 * (k + 1), :])
                else:
                    es, h = item[1], item[2]
                    nc.sync.dma_start(
                        inc_t[es][h][:],
                        inc_re[:, MH * h:MH * (h + 1),
                               128 * es:128 * (es + 1)])

            # device precompute: W2 = ec_W_proj @ fc_W, b2 = ecb@fcW + fcb
            def precompute_w2_b2():
                for j, (cj, oj) in enumerate(zip(DCH, DOF)):
                    w2p = pps.tile([128, NCAT], F32, tag="small", name=f"w2p{j}")
                    for i, (ci, _) in enumerate(zip(DCH, DOF)):
                        nc.tensor.matmul(w2p[:cj, :],
                                         ecpT_sb[:ci, i, oj:oj + cj],
                                         fcw_sb[:ci, i, :],
                                         start=(i == 0), stop=(i == 2))
                    nc.scalar.copy(w2_sb[:cj, j, :], w2p[:cj, :])
                b2p = pps.tile([1, NCAT], F32, tag="small", name="b2p")
                for i, (ci, _) in enumerate(zip(DCH, DOF)):
                    nc.tensor.matmul(b2p[:], ecbc_sb[:ci, i, :],
                                     fcw_sb[:ci, i, :],
                                     start=(i == 0), stop=(i == 2))
                nc.vector.tensor_add(b2_sb[:, 0:NCAT], b2p[:], fcb_sb[:])

            ix_of = {}
            ef_of = {}
            acc_of = {}

            def sweep(es, h):
                if h == 0:
                    acc_of[es] = pp1.tile([128, D], F32, tag="p1", name=f"acc{es}")
                acc = acc_of[es]
                for mt in range(MH):
                    nc.tensor.matmul(acc[:], inc_t[es][h][:, mt, :],
                                     x_sb[:, MH * h + mt, :],
                                     start=(h == 0 and mt == 0),
                                     stop=(h == 1 and mt == MH - 1))

            def tail_copy(es):
                # psum -> sbuf (frees the phase-1 psum for es+2)
                ix = ixp.tile([128, D], F32, tag="ix", name=f"ix{es}")
                ix_of[es] = ix
                nc.vector.tensor_copy(ix[:], acc_of[es][:])

            def tail_pe1(es):
                # transpose IX block -> ixT ; att matmul (f32r)
                ix = ix_of[es]
                for i, (c, o) in enumerate(zip(DCH, DOF)):
                    tp = ppt.tile([128, 128], F32, tag="tp", name=f"tp{es}_{i}")
                    nc.tensor.transpose(tp[:c, :], ix[:, o:o + c], ident[:])
                    nc.scalar.copy(ixT_sb[:c, i,
                                          128 * es:128 * (es + 1)].bitcast(F32R),
                                   tp[:c, :])
                att = ppa.tile([128, D], F32, tag="att", name=f"att{es}")
                for i, (c, _) in enumerate(zip(DCH, DOF)):
                    nc.tensor.matmul(att[:],
                                     r(ixT_sb[:c, i,
                                              128 * es:128 * (es + 1)]),
                                     r(watt_sb[:c, i, :]),
                                     start=(i == 0), stop=(i == 2))
                return att

            def tail_soft(es, att):
                # rowwise softmax over d ; ef = attn * IX
                nmax = stat_sb[:, es, 0:1]
                nc.vector.tensor_reduce(nmax, att[:], axis=AX.X, op=OP.max,
                                        negate=True)
                ex = exp_p.tile([128, D], F32, tag="ex", name=f"ex{es}")
                rsum = stat_sb[:, es, 1:2]
                nc.scalar.activation(ex[:], att[:], AF.Exp, bias=nmax,
                                     scale=1.0, accum_out=rsum)
                rcp = stat_sb[:, es, 2:3]
                nc.vector.reciprocal(rcp, rsum)
                ef = efp.tile([128, D], F32, tag="ef", name=f"ef{es}")
                ef_of[es] = ef
                nc.vector.scalar_tensor_tensor(
                    ef[:], ex[:], rcp, ix_of[es][:], op0=OP.mult,
                    op1=OP.mult)

            def tail_pe2(es):
                ef = ef_of[es]
                for i, (c, o) in enumerate(zip(DCH, DOF)):
                    tp = ppt.tile([128, 128], F32, tag="tp", name=f"tp{es}_{i}")
                    nc.tensor.transpose(tp[:c, :], ef[:, o:o + c], ident[:])
                    nc.scalar.copy(efT_sb[:c, i,
                                          128 * es:128 * (es + 1)].bitcast(F32R),
                                   tp[:c, :])

            atts = {}
            for es in range(ET):
                sweep(es, 0)
                if es == 0:
                    precompute_w2_b2()
                if es >= 1:
                    atts[es - 1] = tail_pe1(es - 1)
                    tail_soft(es - 1, atts[es - 1])
                sweep(es, 1)
                tail_copy(es)
                if es >= 2:
                    tail_pe2(es - 2)
            atts[ET - 1] = tail_pe1(ET - 1)
            tail_soft(ET - 1, atts[ET - 1])
            tail_pe2(ET - 2)
            tail_pe2(ET - 1)

            # ---------- whole-shard tail ----------
            # ef2T = alpha*eftT + (1-alpha) * (W_proj.T @ efT)
            for j, (cj, oj) in enumerate(zip(DCH, DOF)):
                prj = ppj.tile([128, E_SH], F32, tag="prj", name=f"prj{j}")
                for i, (ci, _) in enumerate(zip(DCH, DOF)):
                    nc.tensor.matmul(prj[:cj, :],
                                     r(wproj_sb[:ci, i, oj:oj + cj]),
                                     r(efT_sb[:ci, i, :]),
                                     start=(i == 0), stop=(i == 2))
                nc.vector.scalar_tensor_tensor(
                    ef2T_sb[:cj, j, :].bitcast(F32R), prj[:cj, :],
                    float(1.0 - alpha), efs_sb[:cj, j, :], op0=OP.mult,
                    op1=OP.add)

            # scores (1, 512) ; unstabilized exp weights (|sc| < 6)
            sc = ppj.tile([1, E_SH], F32, tag="prj", name="sc")
            for i, (ci, _) in enumerate(zip(DCH, DOF)):
                nc.tensor.matmul(sc[:], r(ecwatt_sb[:ci, i, :]),
                                 r(ef2T_sb[:ci, i, :]),
                                 start=(i == 0), stop=(i == 2))
            expw = expw_sb[:, 0:E_SH]
            z = expw_sb[:, E_SH:E_SH + 1]
            nc.scalar.activation(expw, sc[:], AF.Exp, bias=0.0, scale=1.0,
                                 accum_out=z)

            # expw columns (PE transpose per 128 block)
            for es in range(ET):
                tc1 = ppt.tile([128, 128], F32, tag="tp", name=f"tc1_{es}")
                nc.tensor.transpose(tc1[:, 0:1],
                                    expw[0:1, 128 * es:128 * (es + 1)],
                                    ident[0:1, 0:1])
                nc.scalar.copy(expcol_sb[:, es:es + 1], tc1[:, 0:1])

            # G2 = ef2 @ W2 (e-partitioned out) ; p2 = expw.T @ G2
            for es in range(ET):
                g2 = pps.tile([128, NCAT], F32, tag="small", name=f"g2_{es}")
                for i, (ci, _) in enumerate(zip(DCH, DOF)):
                    nc.tensor.matmul(g2[:],
                                     ef2T_sb[:ci, i,
                                             128 * es:128 * (es + 1)],
                                     w2_sb[:ci, i, :],
                                     start=(i == 0), stop=(i == 2))
                nc.scalar.copy(g2_sb[:, es, :], g2[:])
            p2 = pps.tile([1, NCAT], F32, tag="small", name="p2")
            for es in range(ET):
                nc.tensor.matmul(p2[:], expcol_sb[:, es:es + 1],
                                 g2_sb[:, es, :], start=(es == 0),
                                 stop=(es == ET - 1))

            nc.scalar.copy(prt_sb[:, 0:NCAT], p2[:])
            nc.scalar.copy(prt_sb[:, NCAT:NCAT + 1], z)
            nc.sync.dma_start(prt_d[:], prt_sb[0:1, :])

            # ---------- AllReduce + tiny epilogue ----------
            nc.gpsimd.collective_compute(
                "AllReduce", OP.add, replica_groups=groups,
                ins=[prt_d.opt()], outs=[cmb_d.opt()])
            nc.sync.dma_start(cmb_sb[:],
                              cmb_d[:].rearrange("(o k) -> o k", o=1))
            rz = expw_sb[:, E_SH + 1:E_SH + 2]
            nc.vector.reciprocal(rz, cmb_sb[:, NCAT:NCAT + 1])
            nc.vector.scalar_tensor_tensor(
                logit_sb[:], cmb_sb[:, 0:NCAT], rz, b2_sb[:, 0:NCAT],
                op0=OP.mult, op1=OP.add)
            nc.sync.dma_start(out_d[:], logit_sb[:])

    nc.compile()
    return nc


_CACHE = {}


def get_nc(alpha: float):
    if alpha not in _CACHE:
        _CACHE[alpha] = _build(alpha)
    return _CACHE[alpha]


def make_in_maps(node_feats, edge_feats, inc_mat, W_att, W_proj,
                 ec_W_att, ec_W_proj, ec_b_proj, fc_W, fc_b):
    import ml_dtypes
    cc = lambda a: np.ascontiguousarray(np.asarray(a, np.float32))
    x_bf = np.ascontiguousarray(
        np.asarray(node_feats, np.float32).astype(ml_dtypes.bfloat16))
    inc_f = np.asarray(inc_mat, np.float32)
    eft = np.asarray(edge_feats, np.float32).T  # (D, E)
    common = dict(watt=cc(W_att), wproj=cc(W_proj),
                  ecwatt=cc(ec_W_att).reshape(D, 1),
                  ecpt=cc(np.asarray(ec_W_proj, np.float32).T),
                  ecb=cc(ec_b_proj), fcw=cc(fc_W), fcb=cc(fc_b))
    in_maps = []
    for c in range(NCORES):
        sl = slice(E_SH * c, E_SH * (c + 1))
        in_maps.append(dict(
            x=x_bf,
            inc=np.ascontiguousarray(inc_f[:, sl].astype(ml_dtypes.bfloat16)),
            eft=np.ascontiguousarray(eft[:, sl]),
            **common))
    return in_maps


def kernel(node_feats, edge_feats, inc_mat, W_att, W_proj, alpha,
           ec_W_att, ec_W_proj, ec_b_proj, fc_W, fc_b, trace=False,
           mode=None):
    nc = get_nc(float(np.asarray(alpha)))
    in_maps = make_in_maps(node_feats, edge_feats, inc_mat, W_att, W_proj,
                           ec_W_att, ec_W_proj, ec_b_proj, fc_W, fc_b)
    res = run_bass_kernel_spmd(nc, in_maps, list(range(NCORES)), trace=trace)
    kernel.last_results = res
    return res.results[0]["out"].reshape(NCAT).astype(np.float32)


# revision 12
# speedup vs baseline: 1.3053x; 1.2490x over previous
"""HGConv fused kernel for one TRN2 chip (8 NeuronCores), SPMD via Bass/Tile.

Hardcoded for M=16384 nodes, E=4096 hyperedges, D=300, N_CAT=3, 8 cores.

Edge-sharded design (v3) — no mid-kernel collectives:
  - Core c owns edges Ec = [512c, 512(c+1)).  Inputs per core: full X and
    inc[:, Ec] in bf16, HOST-PREPACKED into partition-major layout
    [128, t*d] so every DMA line is a multi-KB contiguous row;
    edge_feats[Ec].T (f32, host-transposed); small weights.
  - Phase 1 computes IX_c = inc_c.T @ X over ALL 16384 nodes, m-major
    (x and inc tiles stream through small pools; 4 psum banks accumulate
    the 4 x 128-edge sub-blocks).
  - Tail (batched, stage-major): IX -> transpose -> edge_att = IX@W_att
    (f32r), rowwise softmax over d, ef = attn*IX, transpose, ef2T =
    alpha*eftT + (1-alpha)*(W_proj.T @ efT), scores = ec_W_att.T @ ef2T
    (|scores| < 5 so exp is unstabilized), G2 = ef2 @ (ec_W_proj@fc_W),
    p2 = expw.T @ G2, z = sum(expw).
  - One 4-float AllGather; every core combines the 8 partials with a
    ones-vector matmul and emits logits = p2/z + (ecb@fcW + fcb).
"""

import sys

for _p in ("/opt/trn_rl_repo", "/opt/pypackages"):
    if _p not in sys.path:
        sys.path.append(_p)

import numpy as np

import concourse.bacc as bacc
import concourse.tile as tile
from concourse import masks, mybir
from concourse.bass_utils import run_bass_kernel_spmd

F32 = mybir.dt.float32
F32R = mybir.dt.float32r
BF16 = mybir.dt.bfloat16
AX = mybir.AxisListType
OP = mybir.AluOpType
AF = mybir.ActivationFunctionType

NCORES = 8
M, E, D, NCAT = 16384, 4096, 300, 3
E_SH = E // NCORES          # 512 edges per core
ET = E_SH // 128            # 4 e-sub-blocks per core
MT = M // 128               # 128 m-tiles
MG = 16                     # m-tiles per streamed group
NG = MT // MG               # 8 groups
DCH = (128, 128, 44)        # d split into partition chunks
DOF = (0, 128, 256)


def _build(alpha: float):
    nc = bacc.Bacc("TRN2", target_bir_lowering=False, debug=False,
                   num_devices=NCORES)
    # prepacked [128, t*d] partition-major layouts
    xp_d = nc.dram_tensor("xp", [128, MT * D], BF16, kind="ExternalInput")
    incp_d = nc.dram_tensor("incp", [128, MT * E_SH], BF16,
                            kind="ExternalInput")
    eft_d = nc.dram_tensor("eft", [D, E_SH], F32, kind="ExternalInput")
    watt_d = nc.dram_tensor("watt", [D, D], F32, kind="ExternalInput")
    wproj_d = nc.dram_tensor("wproj", [D, D], F32, kind="ExternalInput")
    ecwatt_d = nc.dram_tensor("ecwatt", [D, 1], F32, kind="ExternalInput")
    # ec_W_proj passed TRANSPOSED from host (only used via W2 = ecp @ fcw)
    ecpT_d = nc.dram_tensor("ecpt", [D, D], F32, kind="ExternalInput")
    ecb_d = nc.dram_tensor("ecb", [D], F32, kind="ExternalInput")
    fcw_d = nc.dram_tensor("fcw", [D, NCAT], F32, kind="ExternalInput")
    fcb_d = nc.dram_tensor("fcb", [NCAT], F32, kind="ExternalInput")
    out_d = nc.dram_tensor("out", [1, NCAT], F32, kind="ExternalOutput")

    groups = [list(range(NCORES))]

    def r(ap):
        return ap.bitcast(F32R)

    with tile.TileContext(nc) as tc, \
         tc.tile_pool(name="sb", bufs=1) as sb, \
         tc.tile_pool(name="dram", bufs=1, space="DRAM") as dram:

        prt_d = dram.tile([4], F32)            # AllGather input  [p2, z]
        gat_d = dram.tile([NCORES * 4], F32)   # AllGather output

        # ---------- small weight loads (Act + Pool rings) ----------
        watt_sb = sb.tile([128, 3, D], F32)
        wproj_sb = sb.tile([128, 3, D], F32)
        ecpT_sb = sb.tile([128, 3, D], F32)
        fcw_sb = sb.tile([128, 3, NCAT], F32)
        ecwatt_sb = sb.tile([128, 3, 1], F32)
        ecbc_sb = sb.tile([128, 3, 1], F32)
        eft_sb = sb.tile([128, 3, E_SH], F32)
        efs_sb = sb.tile([128, 3, E_SH], F32)
        for i, (c, o) in enumerate(zip(DCH, DOF)):
            nc.scalar.dma_start(watt_sb[:c, i, :].bitcast(F32R),
                                watt_d[o:o + c, :].bitcast(F32R))
            nc.scalar.dma_start(wproj_sb[:c, i, :].bitcast(F32R),
                                wproj_d[o:o + c, :].bitcast(F32R))
            nc.scalar.dma_start(eft_sb[:c, i, :], eft_d[o:o + c, :])
            nc.gpsimd.dma_start(ecpT_sb[:c, i, :], ecpT_d[o:o + c, :])
            nc.gpsimd.dma_start(fcw_sb[:c, i, :], fcw_d[o:o + c, :])
            nc.gpsimd.dma_start(ecwatt_sb[:c, i, :].bitcast(F32R),
                                ecwatt_d[o:o + c, :].bitcast(F32R))
            nc.gpsimd.dma_start(
                ecbc_sb[:c, i, 0:1],
                ecb_d[o:o + c].rearrange("(p o) -> p o", o=1))
        fcb_sb = sb.tile([1, NCAT], F32)
        nc.gpsimd.dma_start(fcb_sb[:], fcb_d.ap().rearrange("(o d) -> o d",
                                                            o=1))
        ident = sb.tile([128, 128], F32)
        masks.make_identity(nc, ident[:])
        ones8_sb = sb.tile([NCORES, 1], F32)
        nc.vector.memset(ones8_sb[:], 1.0)

        # efs = alpha * edge_feats.T  (d-partitioned), overlapped w/ phase 1
        for i, (c, o) in enumerate(zip(DCH, DOF)):
            nc.scalar.mul(efs_sb[:c, i, :], eft_sb[:c, i, :], float(alpha))

        # persistent tail state
        ix_sb = sb.tile([128, ET, D], F32)
        ex_sb = sb.tile([128, ET, D], F32)
        ef_sb = sb.tile([128, ET, D], F32)
        ixT_sb = sb.tile([128, 3, E_SH], F32)
        efT_sb = sb.tile([128, 3, E_SH], F32)
        ef2T_sb = sb.tile([128, 3, E_SH], F32)
        w2_sb = sb.tile([128, 3, NCAT], F32)
        stat_sb = sb.tile([128, ET, 4], F32)
        expw_sb = sb.tile([1, E_SH + 4], F32)
        expcol_sb = sb.tile([128, ET], F32)
        g2_sb = sb.tile([128, ET, NCAT], F32)
        b2_sb = sb.tile([1, 4], F32)
        prt_sb = sb.tile([1, 4], F32)
        g8_sb = sb.tile([NCORES, 4], F32)
        cmb_sb = sb.tile([1, 4], F32)
        logit_sb = sb.tile([1, NCAT], F32)

        # ---------- phase 1: m-major streamed IX = inc.T @ X ----------
        with tc.tile_pool(name="xpool", bufs=2) as xpool, \
             tc.tile_pool(name="incpool", bufs=3) as incpool, \
             tc.tile_pool(name="pp1", bufs=4, space="PSUM") as pp1, \
             tc.tile_pool(name="ppw", bufs=1, space="PSUM") as ppw:

            accs = [pp1.tile([128, D], F32, tag="p1", name=f"acc{es}")
                    for es in range(ET)]

            for g in range(NG):
                xt = xpool.tile([128, MG * D], BF16, tag="x", name=f"x{g}")
                it = incpool.tile([128, MG * E_SH], BF16, tag="inc",
                                  name=f"inc{g}")
                nc.sync.dma_start(xt[:], xp_d[:, MG * D * g:MG * D * (g + 1)])
                nc.sync.dma_start(it[:],
                                  incp_d[:, MG * E_SH * g:
                                         MG * E_SH * (g + 1)])
                for mt in range(MG):
                    for es in range(ET):
                        nc.tensor.matmul(
                            accs[es][:],
                            it[:, mt * E_SH + 128 * es:
                               mt * E_SH + 128 * (es + 1)],
                            xt[:, mt * D:(mt + 1) * D],
                            start=(g == 0 and mt == 0),
                            stop=(g == NG - 1 and mt == MG - 1))
                if g == 0:
                    # device precompute, hidden under phase 1:
                    # W2 = ec_W_proj @ fc_W ; b2 = ecb @ fcW + fcb
                    for j, (cj, oj) in enumerate(zip(DCH, DOF)):
                        w2p = ppw.tile([128, NCAT], F32, tag="w",
                                       name=f"w2p{j}")
                        for i, (ci, _) in enumerate(zip(DCH, DOF)):
                            nc.tensor.matmul(w2p[:cj, :],
                                             ecpT_sb[:ci, i, oj:oj + cj],
                                             fcw_sb[:ci, i, :],
                                             start=(i == 0), stop=(i == 2))
                        nc.scalar.copy(w2_sb[:cj, j, :], w2p[:cj, :])
                    b2p = ppw.tile([1, NCAT], F32, tag="w", name="b2p")
                    for i, (ci, _) in enumerate(zip(DCH, DOF)):
                        nc.tensor.matmul(b2p[:], ecbc_sb[:ci, i, :],
                                         fcw_sb[:ci, i, :],
                                         start=(i == 0), stop=(i == 2))
                    nc.vector.tensor_add(b2_sb[:, 0:NCAT], b2p[:],
                                         fcb_sb[:])

            # psum -> sbuf (inside pp1 scope)
            for es in range(ET):
                nc.vector.tensor_copy(ix_sb[:, es, :], accs[es][:])

        # ---------- batched tail ----------
        with tc.tile_pool(name="ppt", bufs=3, space="PSUM") as ppt, \
             tc.tile_pool(name="ppa", bufs=2, space="PSUM") as ppa, \
             tc.tile_pool(name="ppj", bufs=2, space="PSUM") as ppj, \
             tc.tile_pool(name="pps", bufs=1, space="PSUM") as pps:

            # stage 1+2 per es: IX -> ixT ; att (f32r) ; softmax ; ef
            for es in range(ET):
                for i, (c, o) in enumerate(zip(DCH, DOF)):
                    tp = ppt.tile([128, 128], F32, tag="tp",
                                  name=f"tpa_{es}_{i}")
                    nc.tensor.transpose(tp[:c, :], ix_sb[:, es, o:o + c],
                                        ident[:])
                    nc.scalar.copy(
                        ixT_sb[:c, i,
                               128 * es:128 * (es + 1)].bitcast(F32R),
                        tp[:c, :])
                att = ppa.tile([128, D], F32, tag="att", name=f"att{es}")
                for i, (c, _) in enumerate(zip(DCH, DOF)):
                    nc.tensor.matmul(att[:],
                                     r(ixT_sb[:c, i,
                                              128 * es:128 * (es + 1)]),
                                     r(watt_sb[:c, i, :]),
                                     start=(i == 0), stop=(i == 2))
                nmax = stat_sb[:, es, 0:1]
                nc.vector.tensor_reduce(nmax, att[:], axis=AX.X, op=OP.max,
                                        negate=True)
                rsum = stat_sb[:, es, 1:2]
                nc.scalar.activation(ex_sb[:, es, :], att[:], AF.Exp,
                                     bias=nmax, scale=1.0, accum_out=rsum)
                rcp = stat_sb[:, es, 2:3]
                nc.vector.reciprocal(rcp, rsum)
                nc.vector.scalar_tensor_tensor(
                    ef_sb[:, es, :], ex_sb[:, es, :], rcp, ix_sb[:, es, :],
                    op0=OP.mult, op1=OP.mult)

            # stage 3: ef -> efT
            for es in range(ET):
                for i, (c, o) in enumerate(zip(DCH, DOF)):
                    tp = ppt.tile([128, 128], F32, tag="tp",
                                  name=f"tpe_{es}_{i}")
                    nc.tensor.transpose(tp[:c, :], ef_sb[:, es, o:o + c],
                                        ident[:])
                    nc.scalar.copy(
                        efT_sb[:c, i,
                               128 * es:128 * (es + 1)].bitcast(F32R),
                        tp[:c, :])

            # stage 4: ef2T = alpha*eftT + (1-alpha) * (W_proj.T @ efT)
            for j, (cj, oj) in enumerate(zip(DCH, DOF)):
                prj = ppj.tile([128, E_SH], F32, tag="prj", name=f"prj{j}")
                for i, (ci, _) in enumerate(zip(DCH, DOF)):
                    nc.tensor.matmul(prj[:cj, :],
                                     r(wproj_sb[:ci, i, oj:oj + cj]),
                                     r(efT_sb[:ci, i, :]),
                                     start=(i == 0), stop=(i == 2))
                nc.vector.scalar_tensor_tensor(
                    ef2T_sb[:cj, j, :].bitcast(F32R), prj[:cj, :],
                    float(1.0 - alpha), efs_sb[:cj, j, :], op0=OP.mult,
                    op1=OP.add)

            # stage 5: scores + unstabilized exp weights
            sc = ppj.tile([1, E_SH], F32, tag="prj", name="sc")
            for i, (ci, _) in enumerate(zip(DCH, DOF)):
                nc.tensor.matmul(sc[:], r(ecwatt_sb[:ci, i, :]),
                                 r(ef2T_sb[:ci, i, :]),
                                 start=(i == 0), stop=(i == 2))
            expw = expw_sb[:, 0:E_SH]
            z = expw_sb[:, E_SH:E_SH + 1]
            nc.scalar.activation(expw, sc[:], AF.Exp, bias=0.0, scale=1.0,
                                 accum_out=z)
            for es in range(ET):
                tc1 = ppt.tile([128, 128], F32, tag="tp", name=f"tc1_{es}")
                nc.tensor.transpose(tc1[:, 0:1],
                                    expw[0:1, 128 * es:128 * (es + 1)],
                                    ident[0:1, 0:1])
                nc.scalar.copy(expcol_sb[:, es:es + 1], tc1[:, 0:1])

            # stage 6: G2 = ef2 @ W2 ; p2 = expw.T @ G2
            for es in range(ET):
                g2 = pps.tile([128, NCAT], F32, tag="small", name=f"g2_{es}")
                for i, (ci, _) in enumerate(zip(DCH, DOF)):
                    nc.tensor.matmul(g2[:],
                                     ef2T_sb[:ci, i,
                                             128 * es:128 * (es + 1)],
                                     w2_sb[:ci, i, :],
                                     start=(i == 0), stop=(i == 2))
                nc.scalar.copy(g2_sb[:, es, :], g2[:])
            p2 = pps.tile([1, NCAT], F32, tag="small", name="p2")
            for es in range(ET):
                nc.tensor.matmul(p2[:], expcol_sb[:, es:es + 1],
                                 g2_sb[:, es, :], start=(es == 0),
                                 stop=(es == ET - 1))

            nc.scalar.copy(prt_sb[:, 0:NCAT], p2[:])
            nc.scalar.copy(prt_sb[:, NCAT:NCAT + 1], z)
            nc.sync.dma_start(prt_d[:], prt_sb[0:1, :])

            # ---------- AllGather + tiny epilogue ----------
            nc.gpsimd.collective_compute(
                "AllGather", OP.bypass, replica_groups=groups,
                ins=[prt_d.opt()], outs=[gat_d.opt()])
            nc.sync.dma_start(g8_sb[:],
                              gat_d[:].rearrange("(c k) -> c k", c=NCORES))
            cmb = pps.tile([1, 4], F32, tag="small", name="cmb")
            nc.tensor.matmul(cmb[:], ones8_sb[:], g8_sb[:], start=True,
                             stop=True)
            nc.vector.tensor_copy(cmb_sb[:], cmb[:])
            rz = expw_sb[:, E_SH + 1:E_SH + 2]
            nc.vector.reciprocal(rz, cmb_sb[:, NCAT:NCAT + 1])
            nc.vector.scalar_tensor_tensor(
                logit_sb[:], cmb_sb[:, 0:NCAT], rz, b2_sb[:, 0:NCAT],
                op0=OP.mult, op1=OP.add)
            nc.sync.dma_start(out_d[:], logit_sb[:])

    nc.compile()
    return nc


_CACHE = {}


def get_nc(alpha: float):
    if alpha not in _CACHE:
        _CACHE[alpha] = _build(alpha)
    return _CACHE[alpha]


def _pack(a2d, rows, width):
    # (rows*128, width) row-major -> (128, rows*width) partition-major
    return np.ascontiguousarray(
        a2d.reshape(rows, 128, width).transpose(1, 0, 2).reshape(
            128, rows * width))


def make_in_maps(node_feats, edge_feats, inc_mat, W_att, W_proj,
                 ec_W_att, ec_W_proj, ec_b_proj, fc_W, fc_b):
    import ml_dtypes
    cc = lambda a: np.ascontiguousarray(np.asarray(a, np.float32))
    x_bf = np.asarray(node_feats, np.float32).astype(ml_dtypes.bfloat16)
    xp = _pack(x_bf, MT, D)
    inc_f = np.asarray(inc_mat, np.float32)
    eft = np.asarray(edge_feats, np.float32).T  # (D, E)
    common = dict(watt=cc(W_att), wproj=cc(W_proj),
                  ecwatt=cc(ec_W_att).reshape(D, 1),
                  ecpt=cc(np.asarray(ec_W_proj, np.float32).T),
                  ecb=cc(ec_b_proj), fcw=cc(fc_W), fcb=cc(fc_b))
    in_maps = []
    for c in range(NCORES):
        sl = slice(E_SH * c, E_SH * (c + 1))
        inc_bf = inc_f[:, sl].astype(ml_dtypes.bfloat16)
        in_maps.append(dict(
            xp=xp,
            incp=_pack(inc_bf, MT, E_SH),
            eft=np.ascontiguousarray(eft[:, sl]),
            **common))
    return in_maps


def kernel(node_feats, edge_feats, inc_mat, W_att, W_proj, alpha,
           ec_W_att, ec_W_proj, ec_b_proj, fc_W, fc_b, trace=False,
           mode=None):
    nc = get_nc(float(np.asarray(alpha)))
    in_maps = make_in_maps(node_feats, edge_feats, inc_mat, W_att, W_proj,
                           ec_W_att, ec_W_proj, ec_b_proj, fc_W, fc_b)
    res = run_bass_kernel_spmd(nc, in_maps, list(range(NCORES)), trace=trace)
    kernel.last_results = res
    return res.results[0]["out"].reshape(NCAT).astype(np.float32)


# revision 13
# speedup vs baseline: 1.4200x; 1.0879x over previous
"""HGConv fused kernel for one TRN2 chip (8 NeuronCores), SPMD via Bass/Tile.

Hardcoded for M=16384 nodes, E=4096 hyperedges, D=300, N_CAT=3, 8 cores.

Edge-sharded design (v3) — no mid-kernel collectives:
  - Core c owns edges Ec = [512c, 512(c+1)).  Inputs per core: full X and
    inc[:, Ec] in bf16, HOST-PREPACKED into partition-major layout
    [128, t*d] so every DMA line is a multi-KB contiguous row;
    edge_feats[Ec].T (f32, host-transposed); small weights.
  - Phase 1 computes IX_c = inc_c.T @ X over ALL 16384 nodes, m-major
    (x and inc tiles stream through small pools; 4 psum banks accumulate
    the 4 x 128-edge sub-blocks).
  - Tail (batched, stage-major): IX -> transpose -> edge_att = IX@W_att
    (f32r), rowwise softmax over d, ef = attn*IX, transpose, ef2T =
    alpha*eftT + (1-alpha)*(W_proj.T @ efT), scores = ec_W_att.T @ ef2T
    (|scores| < 5 so exp is unstabilized), G2 = ef2 @ (ec_W_proj@fc_W),
    p2 = expw.T @ G2, z = sum(expw).
  - One 4-float AllGather; every core combines the 8 partials with a
    ones-vector matmul and emits logits = p2/z + (ecb@fcW + fcb).
"""

import sys

for _p in ("/opt/trn_rl_repo", "/opt/pypackages"):
    if _p not in sys.path:
        sys.path.append(_p)

import numpy as np

import concourse.bacc as bacc
import concourse.tile as tile
from concourse import masks, mybir
from concourse.bass_utils import run_bass_kernel_spmd

F32 = mybir.dt.float32
F32R = mybir.dt.float32r
BF16 = mybir.dt.bfloat16
AX = mybir.AxisListType
OP = mybir.AluOpType
AF = mybir.ActivationFunctionType

NCORES = 8
M, E, D, NCAT = 16384, 4096, 300, 3
E_SH = E // NCORES          # 512 edges per core
ET = E_SH // 128            # 4 e-sub-blocks per core
MT = M // 128               # 128 m-tiles
MG = 8                      # m-tiles per streamed group
NG = MT // MG               # 8 groups
DCH = (128, 128, 44)        # d split into partition chunks
DOF = (0, 128, 256)


def _build(alpha: float):
    nc = bacc.Bacc("TRN2", target_bir_lowering=False, debug=False,
                   num_devices=NCORES)
    # prepacked [128, t*d] partition-major layouts
    xp_d = nc.dram_tensor("xp", [128, MT * D], BF16, kind="ExternalInput")
    incp_d = nc.dram_tensor("incp", [128, MT * E_SH], BF16,
                            kind="ExternalInput")
    eft_d = nc.dram_tensor("eft", [D, E_SH], F32, kind="ExternalInput")
    watt_d = nc.dram_tensor("watt", [D, D], F32, kind="ExternalInput")
    wproj_d = nc.dram_tensor("wproj", [D, D], F32, kind="ExternalInput")
    ecwatt_d = nc.dram_tensor("ecwatt", [D, 1], F32, kind="ExternalInput")
    # ec_W_proj passed TRANSPOSED from host (only used via W2 = ecp @ fcw)
    ecpT_d = nc.dram_tensor("ecpt", [D, D], F32, kind="ExternalInput")
    ecb_d = nc.dram_tensor("ecb", [D], F32, kind="ExternalInput")
    fcw_d = nc.dram_tensor("fcw", [D, NCAT], F32, kind="ExternalInput")
    fcb_d = nc.dram_tensor("fcb", [NCAT], F32, kind="ExternalInput")
    out_d = nc.dram_tensor("out", [1, NCAT], F32, kind="ExternalOutput")

    groups = [list(range(NCORES))]

    def r(ap):
        return ap.bitcast(F32R)

    with tile.TileContext(nc) as tc, \
         tc.tile_pool(name="sb", bufs=1) as sb, \
         tc.tile_pool(name="dram", bufs=1, space="DRAM") as dram:

        prt_d = dram.tile([4], F32)            # AllGather input  [p2, z]
        gat_d = dram.tile([NCORES * 4], F32)   # AllGather output
        wrm_d = dram.tile([4], F32)            # warm-up collective in
        wgt_d = dram.tile([NCORES * 4], F32)   # warm-up collective out

        # ---------- small-weight tiles (loads issued mid-phase-1) ----------
        watt_sb = sb.tile([128, 3, D], F32)
        wproj_sb = sb.tile([128, 3, D], F32)
        ecpT_sb = sb.tile([128, 3, D], F32)
        fcw_sb = sb.tile([128, 3, NCAT], F32)
        ecwatt_sb = sb.tile([128, 3, 1], F32)
        ecbc_sb = sb.tile([128, 3, 1], F32)
        eft_sb = sb.tile([128, 3, E_SH], F32)
        efs_sb = sb.tile([128, 3, E_SH], F32)
        fcb_sb = sb.tile([1, NCAT], F32)
        ident = sb.tile([128, 128], F32)
        masks.make_identity(nc, ident[:])
        ones8_sb = sb.tile([NCORES, 1], F32)
        nc.vector.memset(ones8_sb[:], 1.0)

        def load_weights():
            # on the sync ring AFTER the first phase-1 groups so the first
            # matmuls are not delayed by weight traffic
            for i, (c, o) in enumerate(zip(DCH, DOF)):
                nc.sync.dma_start(watt_sb[:c, i, :].bitcast(F32R),
                                  watt_d[o:o + c, :].bitcast(F32R))
                nc.sync.dma_start(wproj_sb[:c, i, :].bitcast(F32R),
                                  wproj_d[o:o + c, :].bitcast(F32R))
                nc.sync.dma_start(eft_sb[:c, i, :], eft_d[o:o + c, :])
                nc.sync.dma_start(ecpT_sb[:c, i, :], ecpT_d[o:o + c, :])
                nc.sync.dma_start(fcw_sb[:c, i, :], fcw_d[o:o + c, :])
                nc.sync.dma_start(ecwatt_sb[:c, i, :].bitcast(F32R),
                                  ecwatt_d[o:o + c, :].bitcast(F32R))
                nc.sync.dma_start(
                    ecbc_sb[:c, i, 0:1],
                    ecb_d[o:o + c].rearrange("(p o) -> p o", o=1))
            nc.sync.dma_start(fcb_sb[:],
                              fcb_d.ap().rearrange("(o d) -> o d", o=1))
            # efs = alpha * edge_feats.T (d-partitioned), hidden in phase 1
            for i, (c, o) in enumerate(zip(DCH, DOF)):
                nc.scalar.mul(efs_sb[:c, i, :], eft_sb[:c, i, :],
                              float(alpha))

        # persistent tail state
        ix_sb = sb.tile([128, ET, D], F32)
        ex_sb = sb.tile([128, ET, D], F32)
        ef_sb = sb.tile([128, ET, D], F32)
        ixT_sb = sb.tile([128, 3, E_SH], F32)
        efT_sb = sb.tile([128, 3, E_SH], F32)
        ef2T_sb = sb.tile([128, 3, E_SH], F32)
        w2_sb = sb.tile([128, 3, NCAT], F32)
        stat_sb = sb.tile([128, ET, 4], F32)
        expw_sb = sb.tile([1, E_SH + 4], F32)
        expcol_sb = sb.tile([128, ET], F32)
        g2_sb = sb.tile([128, ET, NCAT], F32)
        b2_sb = sb.tile([1, 4], F32)
        prt_sb = sb.tile([1, 4], F32)
        g8_sb = sb.tile([NCORES, 4], F32)
        cmb_sb = sb.tile([1, 4], F32)
        logit_sb = sb.tile([1, NCAT], F32)

        # ---------- phase 1: m-major streamed IX = inc.T @ X ----------
        with tc.tile_pool(name="xpool", bufs=4) as xpool, \
             tc.tile_pool(name="incpool", bufs=6) as incpool, \
             tc.tile_pool(name="pp1", bufs=4, space="PSUM") as pp1, \
             tc.tile_pool(name="ppw", bufs=1, space="PSUM") as ppw:

            accs = [pp1.tile([128, D], F32, tag="p1", name=f"acc{es}")
                    for es in range(ET)]

            for g in range(NG):
                xt = xpool.tile([128, MG * D], BF16, tag="x", name=f"x{g}")
                it = incpool.tile([128, MG * E_SH], BF16, tag="inc",
                                  name=f"inc{g}")
                nc.sync.dma_start(xt[:], xp_d[:, MG * D * g:MG * D * (g + 1)])
                nc.sync.dma_start(it[:],
                                  incp_d[:, MG * E_SH * g:
                                         MG * E_SH * (g + 1)])
                for mt in range(MG):
                    for es in range(ET):
                        nc.tensor.matmul(
                            accs[es][:],
                            it[:, mt * E_SH + 128 * es:
                               mt * E_SH + 128 * (es + 1)],
                            xt[:, mt * D:(mt + 1) * D],
                            start=(g == 0 and mt == 0),
                            stop=(g == NG - 1 and mt == MG - 1))
                if g == 0:
                    # warm the collective path early: absorbs CC-engine
                    # cold start + inter-core launch skew while PE works
                    nc.gpsimd.collective_compute(
                        "AllGather", OP.bypass, replica_groups=groups,
                        ins=[wrm_d.opt()], outs=[wgt_d.opt()])
                if g == 3:
                    load_weights()
                if g == 6:
                    # device precompute, hidden under phase 1:
                    # W2 = ec_W_proj @ fc_W ; b2 = ecb @ fcW + fcb
                    for j, (cj, oj) in enumerate(zip(DCH, DOF)):
                        w2p = ppw.tile([128, NCAT], F32, tag="w",
                                       name=f"w2p{j}")
                        for i, (ci, _) in enumerate(zip(DCH, DOF)):
                            nc.tensor.matmul(w2p[:cj, :],
                                             ecpT_sb[:ci, i, oj:oj + cj],
                                             fcw_sb[:ci, i, :],
                                             start=(i == 0), stop=(i == 2))
                        nc.scalar.copy(w2_sb[:cj, j, :], w2p[:cj, :])
                    b2p = ppw.tile([1, NCAT], F32, tag="w", name="b2p")
                    for i, (ci, _) in enumerate(zip(DCH, DOF)):
                        nc.tensor.matmul(b2p[:], ecbc_sb[:ci, i, :],
                                         fcw_sb[:ci, i, :],
                                         start=(i == 0), stop=(i == 2))
                    nc.vector.tensor_add(b2_sb[:, 0:NCAT], b2p[:],
                                         fcb_sb[:])

            # psum -> sbuf (inside pp1 scope)
            for es in range(ET):
                nc.vector.tensor_copy(ix_sb[:, es, :], accs[es][:])

        # ---------- batched tail ----------
        with tc.tile_pool(name="ppt", bufs=3, space="PSUM") as ppt, \
             tc.tile_pool(name="ppa", bufs=2, space="PSUM") as ppa, \
             tc.tile_pool(name="ppj", bufs=2, space="PSUM") as ppj, \
             tc.tile_pool(name="pps", bufs=1, space="PSUM") as pps:

            # stage 1+2 per es: IX -> ixT ; att (f32r) ; softmax ; ef
            for es in range(ET):
                for i, (c, o) in enumerate(zip(DCH, DOF)):
                    tp = ppt.tile([128, 128], F32, tag="tp",
                                  name=f"tpa_{es}_{i}")
                    nc.tensor.transpose(tp[:c, :], ix_sb[:, es, o:o + c],
                                        ident[:])
                    nc.scalar.copy(
                        ixT_sb[:c, i,
                               128 * es:128 * (es + 1)].bitcast(F32R),
                        tp[:c, :])
                att = ppa.tile([128, D], F32, tag="att", name=f"att{es}")
                for i, (c, _) in enumerate(zip(DCH, DOF)):
                    nc.tensor.matmul(att[:],
                                     r(ixT_sb[:c, i,
                                              128 * es:128 * (es + 1)]),
                                     r(watt_sb[:c, i, :]),
                                     start=(i == 0), stop=(i == 2))
                nmax = stat_sb[:, es, 0:1]
                nc.vector.tensor_reduce(nmax, att[:], axis=AX.X, op=OP.max,
                                        negate=True)
                rsum = stat_sb[:, es, 1:2]
                nc.scalar.activation(ex_sb[:, es, :], att[:], AF.Exp,
                                     bias=nmax, scale=1.0, accum_out=rsum)
                rcp = stat_sb[:, es, 2:3]
                nc.vector.reciprocal(rcp, rsum)
                nc.vector.scalar_tensor_tensor(
                    ef_sb[:, es, :], ex_sb[:, es, :], rcp, ix_sb[:, es, :],
                    op0=OP.mult, op1=OP.mult)

            # stage 3: ef -> efT
            for es in range(ET):
                for i, (c, o) in enumerate(zip(DCH, DOF)):
                    tp = ppt.tile([128, 128], F32, tag="tp",
                                  name=f"tpe_{es}_{i}")
                    nc.tensor.transpose(tp[:c, :], ef_sb[:, es, o:o + c],
                                        ident[:])
                    nc.scalar.copy(
                        efT_sb[:c, i,
                               128 * es:128 * (es + 1)].bitcast(F32R),
                        tp[:c, :])

            # stage 4: ef2T = alpha*eftT + (1-alpha) * (W_proj.T @ efT)
            for j, (cj, oj) in enumerate(zip(DCH, DOF)):
                prj = ppj.tile([128, E_SH], F32, tag="prj", name=f"prj{j}")
                for i, (ci, _) in enumerate(zip(DCH, DOF)):
                    nc.tensor.matmul(prj[:cj, :],
                                     r(wproj_sb[:ci, i, oj:oj + cj]),
                                     r(efT_sb[:ci, i, :]),
                                     start=(i == 0), stop=(i == 2))
                nc.vector.scalar_tensor_tensor(
                    ef2T_sb[:cj, j, :].bitcast(F32R), prj[:cj, :],
                    float(1.0 - alpha), efs_sb[:cj, j, :], op0=OP.mult,
                    op1=OP.add)

            # stage 5: scores + unstabilized exp weights
            sc = ppj.tile([1, E_SH], F32, tag="prj", name="sc")
            for i, (ci, _) in enumerate(zip(DCH, DOF)):
                nc.tensor.matmul(sc[:], r(ecwatt_sb[:ci, i, :]),
                                 r(ef2T_sb[:ci, i, :]),
                                 start=(i == 0), stop=(i == 2))
            expw = expw_sb[:, 0:E_SH]
            z = expw_sb[:, E_SH:E_SH + 1]
            nc.scalar.activation(expw, sc[:], AF.Exp, bias=0.0, scale=1.0,
                                 accum_out=z)
            for es in range(ET):
                tc1 = ppt.tile([128, 128], F32, tag="tp", name=f"tc1_{es}")
                nc.tensor.transpose(tc1[:, 0:1],
                                    expw[0:1, 128 * es:128 * (es + 1)],
                                    ident[0:1, 0:1])
                nc.scalar.copy(expcol_sb[:, es:es + 1], tc1[:, 0:1])

            # stage 6: G2 = ef2 @ W2 ; p2 = expw.T @ G2
            for es in range(ET):
                g2 = pps.tile([128, NCAT], F32, tag="small", name=f"g2_{es}")
                for i, (ci, _) in enumerate(zip(DCH, DOF)):
                    nc.tensor.matmul(g2[:],
                                     ef2T_sb[:ci, i,
                                             128 * es:128 * (es + 1)],
                                     w2_sb[:ci, i, :],
                                     start=(i == 0), stop=(i == 2))
                nc.scalar.copy(g2_sb[:, es, :], g2[:])
            p2 = pps.tile([1, NCAT], F32, tag="small", name="p2")
            for es in range(ET):
                nc.tensor.matmul(p2[:], expcol_sb[:, es:es + 1],
                                 g2_sb[:, es, :], start=(es == 0),
                                 stop=(es == ET - 1))

            nc.scalar.copy(prt_sb[:, 0:NCAT], p2[:])
            nc.scalar.copy(prt_sb[:, NCAT:NCAT + 1], z)
            nc.sync.dma_start(prt_d[:], prt_sb[0:1, :])

            # ---------- AllGather + tiny epilogue ----------
            nc.gpsimd.collective_compute(
                "AllGather", OP.bypass, replica_groups=groups,
                ins=[prt_d.opt()], outs=[gat_d.opt()])
            nc.sync.dma_start(g8_sb[:],
                              gat_d[:].rearrange("(c k) -> c k", c=NCORES))
            cmb = pps.tile([1, 4], F32, tag="small", name="cmb")
            nc.tensor.matmul(cmb[:], ones8_sb[:], g8_sb[:], start=True,
                             stop=True)
            nc.vector.tensor_copy(cmb_sb[:], cmb[:])
            rz = expw_sb[:, E_SH + 1:E_SH + 2]
            nc.vector.reciprocal(rz, cmb_sb[:, NCAT:NCAT + 1])
            nc.vector.scalar_tensor_tensor(
                logit_sb[:], cmb_sb[:, 0:NCAT], rz, b2_sb[:, 0:NCAT],
                op0=OP.mult, op1=OP.add)
            nc.sync.dma_start(out_d[:], logit_sb[:])

    nc.compile()
    return nc


_CACHE = {}


def get_nc(alpha: float):
    if alpha not in _CACHE:
        _CACHE[alpha] = _build(alpha)
    return _CACHE[alpha]


def _pack(a2d, rows, width):
    # (rows*128, width) row-major -> (128, rows*width) partition-major
    return np.ascontiguousarray(
        a2d.reshape(rows, 128, width).transpose(1, 0, 2).reshape(
            128, rows * width))


def make_in_maps(node_feats, edge_feats, inc_mat, W_att, W_proj,
                 ec_W_att, ec_W_proj, ec_b_proj, fc_W, fc_b):
    import ml_dtypes
    cc = lambda a: np.ascontiguousarray(np.asarray(a, np.float32))
    x_bf = np.asarray(node_feats, np.float32).astype(ml_dtypes.bfloat16)
    xp = _pack(x_bf, MT, D)
    inc_f = np.asarray(inc_mat, np.float32)
    eft = np.asarray(edge_feats, np.float32).T  # (D, E)
    common = dict(watt=cc(W_att), wproj=cc(W_proj),
                  ecwatt=cc(ec_W_att).reshape(D, 1),
                  ecpt=cc(np.asarray(ec_W_proj, np.float32).T),
                  ecb=cc(ec_b_proj), fcw=cc(fc_W), fcb=cc(fc_b))
    in_maps = []
    for c in range(NCORES):
        sl = slice(E_SH * c, E_SH * (c + 1))
        inc_bf = inc_f[:, sl].astype(ml_dtypes.bfloat16)
        in_maps.append(dict(
            xp=xp,
            incp=_pack(inc_bf, MT, E_SH),
            eft=np.ascontiguousarray(eft[:, sl]),
            **common))
    return in_maps


def kernel(node_feats, edge_feats, inc_mat, W_att, W_proj, alpha,
           ec_W_att, ec_W_proj, ec_b_proj, fc_W, fc_b, trace=False,
           mode=None):
    nc = get_nc(float(np.asarray(alpha)))
    in_maps = make_in_maps(node_feats, edge_feats, inc_mat, W_att, W_proj,
                           ec_W_att, ec_W_proj, ec_b_proj, fc_W, fc_b)
    res = run_bass_kernel_spmd(nc, in_maps, list(range(NCORES)), trace=trace)
    kernel.last_results = res
    return res.results[0]["out"].reshape(NCAT).astype(np.float32)


# revision 14
# speedup vs baseline: 1.5931x; 1.1219x over previous
"""HGConv fused kernel for one TRN2 chip (8 NeuronCores), SPMD via Bass/Tile.

Hardcoded for M=16384 nodes, E=4096 hyperedges, D=300, N_CAT=3, 8 cores.

Edge-sharded design (v3) — no mid-kernel collectives:
  - Core c owns edges Ec = [512c, 512(c+1)).  Inputs per core: full X and
    inc[:, Ec] in bf16, HOST-PREPACKED into partition-major layout
    [128, t*d] so every DMA line is a multi-KB contiguous row;
    edge_feats[Ec].T (f32, host-transposed); small weights.
  - Phase 1 computes IX_c = inc_c.T @ X over ALL 16384 nodes, m-major
    (x and inc tiles stream through small pools; 4 psum banks accumulate
    the 4 x 128-edge sub-blocks).
  - Tail (batched, stage-major): IX -> transpose -> edge_att = IX@W_att
    (f32r), rowwise softmax over d, ef = attn*IX, transpose, ef2T =
    alpha*eftT + (1-alpha)*(W_proj.T @ efT), scores = ec_W_att.T @ ef2T
    (|scores| < 5 so exp is unstabilized), G2 = ef2 @ (ec_W_proj@fc_W),
    p2 = expw.T @ G2, z = sum(expw).
  - One 4-float AllGather; every core combines the 8 partials with a
    ones-vector matmul and emits logits = p2/z + (ecb@fcW + fcb).
"""

import sys

for _p in ("/opt/trn_rl_repo", "/opt/pypackages"):
    if _p not in sys.path:
        sys.path.append(_p)

import numpy as np

import concourse.bacc as bacc
import concourse.tile as tile
from concourse import masks, mybir
from concourse.bass_utils import run_bass_kernel_spmd

F32 = mybir.dt.float32
F32R = mybir.dt.float32r
BF16 = mybir.dt.bfloat16
AX = mybir.AxisListType
OP = mybir.AluOpType
AF = mybir.ActivationFunctionType

NCORES = 8
M, E, D, NCAT = 16384, 4096, 300, 3
E_SH = E // NCORES          # 512 edges per core
ET = E_SH // 128            # 4 e-sub-blocks per core
MT = M // 128               # 128 m-tiles
MG = 8                      # m-tiles per streamed group
NG = MT // MG               # 8 groups
DCH = (128, 128, 44)        # d split into partition chunks
DOF = (0, 128, 256)


def _build(alpha: float):
    nc = bacc.Bacc("TRN2", target_bir_lowering=False, debug=False,
                   num_devices=NCORES)
    # prepacked [128, t*d] partition-major layouts
    xp_d = nc.dram_tensor("xp", [128, MT * D], BF16, kind="ExternalInput")
    incp_d = nc.dram_tensor("incp", [128, MT * E_SH], BF16,
                            kind="ExternalInput")
    eft_d = nc.dram_tensor("eft", [D, E_SH], F32, kind="ExternalInput")
    watt_d = nc.dram_tensor("watt", [D, D], F32, kind="ExternalInput")
    wproj_d = nc.dram_tensor("wproj", [D, D], F32, kind="ExternalInput")
    ecwatt_d = nc.dram_tensor("ecwatt", [D, 1], F32, kind="ExternalInput")
    # ec_W_proj passed TRANSPOSED from host (only used via W2 = ecp @ fcw)
    ecpT_d = nc.dram_tensor("ecpt", [D, D], F32, kind="ExternalInput")
    ecb_d = nc.dram_tensor("ecb", [D], F32, kind="ExternalInput")
    fcw_d = nc.dram_tensor("fcw", [D, NCAT], F32, kind="ExternalInput")
    fcb_d = nc.dram_tensor("fcb", [NCAT], F32, kind="ExternalInput")
    out_d = nc.dram_tensor("out", [1, NCAT], F32, kind="ExternalOutput")

    groups = [list(range(NCORES))]

    def r(ap):
        return ap.bitcast(F32R)

    with tile.TileContext(nc) as tc, \
         tc.tile_pool(name="sb", bufs=1) as sb, \
         tc.tile_pool(name="dram", bufs=1, space="DRAM") as dram:

        prt_d = dram.tile([4], F32)            # AllGather input  [p2, z]
        gat_d = dram.tile([NCORES * 4], F32)   # AllGather output
        wrm_d = dram.tile([4], F32)            # warm-up collective in
        wgt_d = dram.tile([NCORES * 4], F32)   # warm-up collective out

        # ---------- small-weight tiles (loads issued mid-phase-1) ----------
        watt_sb = sb.tile([128, 3, D], F32)
        wproj_sb = sb.tile([128, 3, D], F32)
        ecpT_sb = sb.tile([128, 3, D], F32)
        fcw_sb = sb.tile([128, 3, NCAT], F32)
        ecwatt_sb = sb.tile([128, 3, 1], F32)
        ecbc_sb = sb.tile([128, 3, 1], F32)
        eft_sb = sb.tile([128, 3, E_SH], F32)
        efs_sb = sb.tile([128, 3, E_SH], F32)
        fcb_sb = sb.tile([1, NCAT], F32)
        ident = sb.tile([128, 128], F32)
        masks.make_identity(nc, ident[:])
        ones8_sb = sb.tile([NCORES, 1], F32)
        nc.vector.memset(ones8_sb[:], 1.0)

        def load_weights(part):
            # on the sync ring AFTER the first phase-1 groups, in small
            # slices so the ring sequencer never starves the input stream
            i, (c, o) = part, (DCH[part % 3], DOF[part % 3])
            if part < 3:
                nc.sync.dma_start(watt_sb[:c, part, :].bitcast(F32R),
                                  watt_d[o:o + c, :].bitcast(F32R))
                nc.sync.dma_start(wproj_sb[:c, part, :].bitcast(F32R),
                                  wproj_d[o:o + c, :].bitcast(F32R))
                nc.sync.dma_start(eft_sb[:c, part, :], eft_d[o:o + c, :])
                nc.scalar.mul(efs_sb[:c, part, :], eft_sb[:c, part, :],
                              float(alpha))
            elif part == 3:
                for i, (c, o) in enumerate(zip(DCH, DOF)):
                    nc.sync.dma_start(ecpT_sb[:c, i, :], ecpT_d[o:o + c, :])
                    nc.sync.dma_start(fcw_sb[:c, i, :], fcw_d[o:o + c, :])
            else:
                for i, (c, o) in enumerate(zip(DCH, DOF)):
                    nc.sync.dma_start(ecwatt_sb[:c, i, :].bitcast(F32R),
                                      ecwatt_d[o:o + c, :].bitcast(F32R))
                    nc.sync.dma_start(
                        ecbc_sb[:c, i, 0:1],
                        ecb_d[o:o + c].rearrange("(p o) -> p o", o=1))
                nc.sync.dma_start(fcb_sb[:],
                                  fcb_d.ap().rearrange("(o d) -> o d", o=1))

        # persistent tail state
        ix_sb = sb.tile([128, ET, D], F32)
        ex_sb = sb.tile([128, ET, D], F32)
        ef_sb = sb.tile([128, ET, D], F32)
        ixT_sb = sb.tile([128, 3, E_SH], F32)
        efT_sb = sb.tile([128, 3, E_SH], F32)
        ef2T_sb = sb.tile([128, 3, E_SH], F32)
        w2_sb = sb.tile([128, 3, NCAT], F32)
        stat_sb = sb.tile([128, ET, 4], F32)
        expw_sb = sb.tile([1, E_SH + 4], F32)
        expcol_sb = sb.tile([128, ET], F32)
        g2_sb = sb.tile([128, ET, NCAT], F32)
        b2_sb = sb.tile([1, 4], F32)
        prt_sb = sb.tile([1, 4], F32)
        g8_sb = sb.tile([NCORES, 4], F32)
        cmb_sb = sb.tile([1, 4], F32)
        logit_sb = sb.tile([1, NCAT], F32)

        # ---------- phase 1: m-major streamed IX = inc.T @ X ----------
        with tc.tile_pool(name="xpool", bufs=4) as xpool, \
             tc.tile_pool(name="incpool", bufs=6) as incpool, \
             tc.tile_pool(name="pp1", bufs=4, space="PSUM") as pp1, \
             tc.tile_pool(name="ppw", bufs=1, space="PSUM") as ppw:

            accs = [pp1.tile([128, D], F32, tag="p1", name=f"acc{es}")
                    for es in range(ET)]

            for g in range(NG):
                xt = xpool.tile([128, MG * D], BF16, tag="x", name=f"x{g}")
                it = incpool.tile([128, MG * E_SH], BF16, tag="inc",
                                  name=f"inc{g}")
                nc.sync.dma_start(xt[:], xp_d[:, MG * D * g:MG * D * (g + 1)])
                nc.sync.dma_start(it[:],
                                  incp_d[:, MG * E_SH * g:
                                         MG * E_SH * (g + 1)])
                for mt in range(MG):
                    for es in range(ET):
                        nc.tensor.matmul(
                            accs[es][:],
                            it[:, mt * E_SH + 128 * es:
                               mt * E_SH + 128 * (es + 1)],
                            xt[:, mt * D:(mt + 1) * D],
                            start=(g == 0 and mt == 0),
                            stop=(g == NG - 1 and mt == MG - 1))
                if g == 0:
                    # warm the collective path early: absorbs CC-engine
                    # cold start + inter-core launch skew while PE works
                    nc.gpsimd.collective_compute(
                        "AllGather", OP.bypass, replica_groups=groups,
                        ins=[wrm_d.opt()], outs=[wgt_d.opt()])
                if g == 5:
                    nc.gpsimd.collective_compute(
                        "AllGather", OP.bypass, replica_groups=groups,
                        ins=[wrm_d.opt()], outs=[wgt_d.opt()])
                if 3 <= g <= 7:
                    load_weights(g - 3)
                if g == 9:
                    # device precompute, hidden under phase 1:
                    # W2 = ec_W_proj @ fc_W ; b2 = ecb @ fcW + fcb
                    for j, (cj, oj) in enumerate(zip(DCH, DOF)):
                        w2p = ppw.tile([128, NCAT], F32, tag="w",
                                       name=f"w2p{j}")
                        for i, (ci, _) in enumerate(zip(DCH, DOF)):
                            nc.tensor.matmul(w2p[:cj, :],
                                             ecpT_sb[:ci, i, oj:oj + cj],
                                             fcw_sb[:ci, i, :],
                                             start=(i == 0), stop=(i == 2))
                        nc.scalar.copy(w2_sb[:cj, j, :], w2p[:cj, :])
                    b2p = ppw.tile([1, NCAT], F32, tag="w", name="b2p")
                    for i, (ci, _) in enumerate(zip(DCH, DOF)):
                        nc.tensor.matmul(b2p[:], ecbc_sb[:ci, i, :],
                                         fcw_sb[:ci, i, :],
                                         start=(i == 0), stop=(i == 2))
                    nc.vector.tensor_add(b2_sb[:, 0:NCAT], b2p[:],
                                         fcb_sb[:])

            # psum -> sbuf (inside pp1 scope)
            for es in range(ET):
                nc.vector.tensor_copy(ix_sb[:, es, :], accs[es][:])

        # ---------- batched tail ----------
        with tc.tile_pool(name="ppt", bufs=3, space="PSUM") as ppt, \
             tc.tile_pool(name="ppa", bufs=2, space="PSUM") as ppa, \
             tc.tile_pool(name="ppj", bufs=2, space="PSUM") as ppj, \
             tc.tile_pool(name="pps", bufs=1, space="PSUM") as pps:

            # stage 1+2 per es: IX -> ixT ; att (f32r) ; softmax ; ef
            for es in range(ET):
                for i, (c, o) in enumerate(zip(DCH, DOF)):
                    tp = ppt.tile([128, 128], F32, tag="tp",
                                  name=f"tpa_{es}_{i}")
                    nc.tensor.transpose(tp[:c, :], ix_sb[:, es, o:o + c],
                                        ident[:])
                    nc.scalar.copy(
                        ixT_sb[:c, i,
                               128 * es:128 * (es + 1)].bitcast(F32R),
                        tp[:c, :])
                att = ppa.tile([128, D], F32, tag="att", name=f"att{es}")
                for i, (c, _) in enumerate(zip(DCH, DOF)):
                    nc.tensor.matmul(att[:],
                                     r(ixT_sb[:c, i,
                                              128 * es:128 * (es + 1)]),
                                     r(watt_sb[:c, i, :]),
                                     start=(i == 0), stop=(i == 2))
                nmax = stat_sb[:, es, 0:1]
                nc.vector.tensor_reduce(nmax, att[:], axis=AX.X, op=OP.max,
                                        negate=True)
                rsum = stat_sb[:, es, 1:2]
                nc.scalar.activation(ex_sb[:, es, :], att[:], AF.Exp,
                                     bias=nmax, scale=1.0, accum_out=rsum)
                rcp = stat_sb[:, es, 2:3]
                nc.vector.reciprocal(rcp, rsum)
                nc.vector.scalar_tensor_tensor(
                    ef_sb[:, es, :], ex_sb[:, es, :], rcp, ix_sb[:, es, :],
                    op0=OP.mult, op1=OP.mult)

            # stage 3: ef -> efT
            for es in range(ET):
                for i, (c, o) in enumerate(zip(DCH, DOF)):
                    tp = ppt.tile([128, 128], F32, tag="tp",
                                  name=f"tpe_{es}_{i}")
                    nc.tensor.transpose(tp[:c, :], ef_sb[:, es, o:o + c],
                                        ident[:])
                    nc.scalar.copy(
                        efT_sb[:c, i,
                               128 * es:128 * (es + 1)].bitcast(F32R),
                        tp[:c, :])

            # stage 4: ef2T = alpha*eftT + (1-alpha) * (W_proj.T @ efT)
            for j, (cj, oj) in enumerate(zip(DCH, DOF)):
                prj = ppj.tile([128, E_SH], F32, tag="prj", name=f"prj{j}")
                for i, (ci, _) in enumerate(zip(DCH, DOF)):
                    nc.tensor.matmul(prj[:cj, :],
                                     r(wproj_sb[:ci, i, oj:oj + cj]),
                                     r(efT_sb[:ci, i, :]),
                                     start=(i == 0), stop=(i == 2))
                nc.vector.scalar_tensor_tensor(
                    ef2T_sb[:cj, j, :].bitcast(F32R), prj[:cj, :],
                    float(1.0 - alpha), efs_sb[:cj, j, :], op0=OP.mult,
                    op1=OP.add)

            # stage 5: scores + unstabilized exp weights
            sc = ppj.tile([1, E_SH], F32, tag="prj", name="sc")
            for i, (ci, _) in enumerate(zip(DCH, DOF)):
                nc.tensor.matmul(sc[:], r(ecwatt_sb[:ci, i, :]),
                                 r(ef2T_sb[:ci, i, :]),
                                 start=(i == 0), stop=(i == 2))
            expw = expw_sb[:, 0:E_SH]
            z = expw_sb[:, E_SH:E_SH + 1]
            nc.scalar.activation(expw, sc[:], AF.Exp, bias=0.0, scale=1.0,
                                 accum_out=z)
            for es in range(ET):
                tc1 = ppt.tile([128, 128], F32, tag="tp", name=f"tc1_{es}")
                nc.tensor.transpose(tc1[:, 0:1],
                                    expw[0:1, 128 * es:128 * (es + 1)],
                                    ident[0:1, 0:1])
                nc.scalar.copy(expcol_sb[:, es:es + 1], tc1[:, 0:1])

            # stage 6: G2 = ef2 @ W2 ; p2 = expw.T @ G2
            for es in range(ET):
                g2 = pps.tile([128, NCAT], F32, tag="small", name=f"g2_{es}")
                for i, (ci, _) in enumerate(zip(DCH, DOF)):
                    nc.tensor.matmul(g2[:],
                                     ef2T_sb[:ci, i,
                                             128 * es:128 * (es + 1)],
                                     w2_sb[:ci, i, :],
                                     start=(i == 0), stop=(i == 2))
                nc.scalar.copy(g2_sb[:, es, :], g2[:])
            p2 = pps.tile([1, NCAT], F32, tag="small", name="p2")
            for es in range(ET):
                nc.tensor.matmul(p2[:], expcol_sb[:, es:es + 1],
                                 g2_sb[:, es, :], start=(es == 0),
                                 stop=(es == ET - 1))

            nc.scalar.copy(prt_sb[:, 0:NCAT], p2[:])
            nc.scalar.copy(prt_sb[:, NCAT:NCAT + 1], z)
            nc.sync.dma_start(prt_d[:], prt_sb[0:1, :])

            # ---------- AllGather + tiny epilogue ----------
            nc.gpsimd.collective_compute(
                "AllGather", OP.bypass, replica_groups=groups,
                ins=[prt_d.opt()], outs=[gat_d.opt()])
            nc.sync.dma_start(g8_sb[:],
                              gat_d[:].rearrange("(c k) -> c k", c=NCORES))
            cmb = pps.tile([1, 4], F32, tag="small", name="cmb")
            nc.tensor.matmul(cmb[:], ones8_sb[:], g8_sb[:], start=True,
                             stop=True)
            nc.vector.tensor_copy(cmb_sb[:], cmb[:])
            rz = expw_sb[:, E_SH + 1:E_SH + 2]
            nc.vector.reciprocal(rz, cmb_sb[:, NCAT:NCAT + 1])
            nc.vector.scalar_tensor_tensor(
                logit_sb[:], cmb_sb[:, 0:NCAT], rz, b2_sb[:, 0:NCAT],
                op0=OP.mult, op1=OP.add)
            nc.sync.dma_start(out_d[:], logit_sb[:])

    nc.compile()
    return nc


_CACHE = {}


def get_nc(alpha: float):
    if alpha not in _CACHE:
        _CACHE[alpha] = _build(alpha)
    return _CACHE[alpha]


def _pack(a2d, rows, width):
    # (rows*128, width) row-major -> (128, rows*width) partition-major
    return np.ascontiguousarray(
        a2d.reshape(rows, 128, width).transpose(1, 0, 2).reshape(
            128, rows * width))


def make_in_maps(node_feats, edge_feats, inc_mat, W_att, W_proj,
                 ec_W_att, ec_W_proj, ec_b_proj, fc_W, fc_b):
    import ml_dtypes
    cc = lambda a: np.ascontiguousarray(np.asarray(a, np.float32))
    x_bf = np.asarray(node_feats, np.float32).astype(ml_dtypes.bfloat16)
    xp = _pack(x_bf, MT, D)
    inc_f = np.asarray(inc_mat, np.float32)
    eft = np.asarray(edge_feats, np.float32).T  # (D, E)
    common = dict(watt=cc(W_att), wproj=cc(W_proj),
                  ecwatt=cc(ec_W_att).reshape(D, 1),
                  ecpt=cc(np.asarray(ec_W_proj, np.float32).T),
                  ecb=cc(ec_b_proj), fcw=cc(fc_W), fcb=cc(fc_b))
    in_maps = []
    for c in range(NCORES):
        sl = slice(E_SH * c, E_SH * (c + 1))
        inc_bf = inc_f[:, sl].astype(ml_dtypes.bfloat16)
        in_maps.append(dict(
            xp=xp,
            incp=_pack(inc_bf, MT, E_SH),
            eft=np.ascontiguousarray(eft[:, sl]),
            **common))
    return in_maps


def kernel(node_feats, edge_feats, inc_mat, W_att, W_proj, alpha,
           ec_W_att, ec_W_proj, ec_b_proj, fc_W, fc_b, trace=False,
           mode=None):
    nc = get_nc(float(np.asarray(alpha)))
    in_maps = make_in_maps(node_feats, edge_feats, inc_mat, W_att, W_proj,
                           ec_W_att, ec_W_proj, ec_b_proj, fc_W, fc_b)
    res = run_bass_kernel_spmd(nc, in_maps, list(range(NCORES)), trace=trace)
    kernel.last_results = res
    return res.results[0]["out"].reshape(NCAT).astype(np.float32)
